# revision 1
# baseline (speedup 1.0000x reference)
"""MemoryTree oracle loss kernel for 8 Trainium2 NeuronCores.

Strategy
--------
reference() computes, per level l, logits[b,k,n] = q[b,k] @ mem_l[b,n] @ v[b,k] / D
where mem_l is the pairwise-mean tree built from `leafs`. Because the logit is
linear in the memory matrix and each parent is the *mean* of its children,
level-l logits are exactly pairwise means of level-0 logits. So the only heavy
work is the leaf-level bilinear forms

    s0[b,k,j] = sum_{d,e} leafs[b,j,d,e] * q[b,k,d] * v[b,k,e] / D

which requires one streaming pass over the 512MB `leafs` tensor (memory-bound).
Everything else (12 levels of log-softmax/NLL/bincount weights over 8x4x4096
floats) is a negligible epilogue done on host in float64.

Device mapping (per core = one batch b), parameterized by QL = consecutive
leaves sharing the partition axis:
  - SBUF data tile partition p = (j_lo in [0,QL)) x (row-group r) over QL
    CONSECUTIVE leaves -> one affine partition dim (stride 32*QL elems).
  - Free dim = (j_hi: leaf-group index, stride QL*4096) x (contiguous burst
    c = (d_lo, e), 32*QL elems).
  - ITERS = 32*QL accumulating matmuls per block, one per (d_lo, e) slice:
    stationary W[(j_lo',r), (j_lo,k)] = delta(j_lo'==j_lo) * q[k,d] * v[k,e]/D
    (host-precomputed, tiny), moving operand = strided slice of the data tile
    (N = 512/QL columns = j_hi). PSUM accumulates the full (d,e) contraction.
  - 8 blocks of 512 leaves, double-buffered 8MB DMAs, one PSUM bank per block.
Output per core: (QL*4, 8*512/QL) = s0 scrambled as [(j_lo,k), (blk,j_hi)].
"""

import os
import sys

import numpy as np

# concourse ships on PYTHONPATH in this environment; add known locations as a
# fallback so kernel.py works from a bare directory.
for _p in ("/root/.axon_site/_ro/trn_rl_repo", "/opt/trn_rl_repo"):
    if _p not in sys.path and os.path.isdir(_p):
        sys.path.append(_p)

B = 8
L_K = 4
D = 64
L = 4096
BLK = 512          # leaves per block
NBLK = L // BLK    # 8


class Cfg:
    def __init__(self, ql: int, data_dt: str, mm_dt: str):
        self.ql = ql                  # consecutive leaves on partition axis
        self.data_dt = data_dt        # dram/sbuf data dtype: 'f32' | 'bf16'
        self.mm_dt = mm_dt            # matmul view dtype: 'f32'|'f32r'|'bf16'
        self.rp = ql // 2 or 1        # d-rows per partition (ql=2 -> 1)
        assert 64 % self.rp == 0 and 128 % ql == 0
        assert ql * (64 // self.rp) == 128  # partitions
        self.iters = self.rp * D      # accumulation steps per block
        self.m = ql * L_K             # stationary free dim / psum partitions
        self.jh = BLK // ql           # moving free dim N
        self.key = f"ql{ql}_{data_dt}_{mm_dt}"

    @property
    def np_data_dt(self):
        if self.data_dt in ("f32", "f32r"):
            return np.float32
        import ml_dtypes
        return ml_dtypes.bfloat16


CFG_A = Cfg(4, "f32", "f32")       # exact fp32 (default)
CFG_B = Cfg(2, "f32r", "f32r")     # relaxed-precision matmul chain, N=256
CFG_F = Cfg(4, "bf16", "bf16")     # bf16 data: half the HBM traffic

# Measured on trn2 (per 64MB pass per core, device time via repeat-slope):
#   CFG_A ~327us  s0 rel err ~5e-7  (end-to-end loss err 0.0 vs f32 reference)
#   CFG_B ~109us  s0 rel err ~1.8e-4 (fp32r truncates to ~13 mantissa bits)
#   CFG_F ~152us  s0 rel err ~2.2e-3
# Default is the exact config; set KERNEL_CFG=f32r|bf16 to trade accuracy for
# speed.
DEFAULT_CFG = {
    "f32": CFG_A, "f32r": CFG_B, "bf16": CFG_F,
}[os.environ.get("KERNEL_CFG", "f32")]

TRACE = False
LAST_EXEC_NS = None
LAST_MEAN_EXEC_NS = None
LAST_PROFILE = None

_PROGRAMS = {}


def _build_program(cfg: Cfg, repeat: int = 1, mode: str = "full"):
    import concourse.bass as bass
    import concourse.tile as tile
    from concourse import bacc, mybir

    f32 = mybir.dt.float32
    ddt = {"f32": f32, "f32r": mybir.dt.float32r,
           "bf16": mybir.dt.bfloat16}[cfg.data_dt]
    mdt = {"f32": f32, "f32r": mybir.dt.float32r,
           "bf16": mybir.dt.bfloat16}[cfg.mm_dt]
    QL, JH, ITERS, M = cfg.ql, cfg.jh, cfg.iters, cfg.m

    nc = bacc.Bacc(None, target_bir_lowering=False, debug=False)
    leafs = nc.declare_dram_parameter("leafs", [L, D, D], ddt, isOutput=False)
    wmat = nc.declare_dram_parameter("wmat", [128, ITERS * M], ddt,
                                     isOutput=False)
    out = nc.declare_dram_parameter("out", [M, NBLK * JH], f32, isOutput=True)

    def mmview(ap):
        return ap if mdt == ddt else ap.bitcast(mdt)

    with tile.TileContext(nc) as tc:
        with (
            tc.tile_pool(name="consts", bufs=1) as consts,
            tc.tile_pool(name="data", bufs=2) as data_pool,
            tc.tile_pool(name="outp", bufs=1) as outp,
            tc.tile_pool(name="psum", bufs=1, space="PSUM") as psum_pool,
        ):
            wt = consts.tile([128, ITERS * M], ddt)
            nc.sync.dma_start(out=wt[:, :], in_=wmat[:, :])
            out_sb = outp.tile([M, NBLK * JH], f32)

            base = leafs[:, :, :]
            pstride = 32 * QL           # partition stride in elements

            # one PSUM bank per block (8 banks exactly) -> maximal overlap.
            ps_list = [
                psum_pool.tile([M, JH], f32, name=f"ps{i}", tag=f"ps{i}")
                for i in range(NBLK)
            ]

            def data_ap(blk):
                return bass.AP(
                    tensor=base.tensor,
                    offset=blk * BLK * D * D,
                    ap=[[pstride, 128], [QL * D * D, JH], [1, ITERS]],
                )

            fixed_dtile = None
            if mode == "mm":
                fixed_dtile = consts.tile([128, JH * ITERS], ddt)
                nc.sync.dma_start(out=fixed_dtile[:, :], in_=data_ap(0))

            for rep in range(repeat):
                for blk in range(NBLK):
                    if mode == "mm":
                        dtile = fixed_dtile
                    else:
                        dtile = data_pool.tile([128, JH * ITERS], ddt)
                        nc.sync.dma_start(out=dtile[:, :], in_=data_ap(blk))
                    ps = ps_list[blk]
                    if mode == "dma":
                        nc.vector.tensor_copy(
                            out=out_sb[0:1, blk * JH:blk * JH + 1],
                            in_=dtile[0:1, 0:1].bitcast(f32)
                            if ddt != f32 else dtile[0:1, 0:1],
                        )
                        continue
                    dview = dtile.rearrange("p (jh c) -> p jh c", c=ITERS)
                    for it in range(ITERS):
                        nc.tensor.matmul(
                            out=ps[:, :],
                            lhsT=mmview(wt[:, it * M:(it + 1) * M]),
                            rhs=mmview(dview[:, :, it]),
                            start=(it == 0),
                            stop=(it == ITERS - 1),
                        )
                    nc.vector.tensor_copy(
                        out=out_sb[:, blk * JH:(blk + 1) * JH], in_=ps[:, :]
                    )

            nc.sync.dma_start(out=out[:, :], in_=out_sb[:, :])

    nc.compile()
    return nc


def _get_program(cfg: Cfg):
    key = cfg.key
    if key not in _PROGRAMS:
        _PROGRAMS[key] = _build_program(cfg)
    return _PROGRAMS[key]


def _build_wmat(cfg: Cfg, qb: np.ndarray, vb: np.ndarray) -> np.ndarray:
    """Stationary weights for one batch: (128, ITERS*M).

    W[p=(j_lo', r), it=(d_lo, e), m=(j_lo, k)]
        = delta(j_lo'==j_lo) * q[k, r*rp + d_lo] * v[k, e] / D
    """
    QL, rp, M, ITERS = cfg.ql, cfg.rp, cfg.m, cfg.iters
    nr = 64 // rp                                   # row-groups per partition
    qv = (qb[:, :, None].astype(np.float64) * vb[:, None, :].astype(np.float64)
          / D).astype(np.float32)                   # (k, d, e)
    rq = qv.reshape(L_K, nr, rp, D)                 # (k, r, d_lo, e)
    rq = np.ascontiguousarray(rq.transpose(1, 2, 3, 0))  # (r, d_lo, e, k)
    w6 = np.zeros((QL, nr, rp, D, QL, L_K), np.float32)
    for jl in range(QL):
        w6[jl, :, :, :, jl, :] = rq
    return np.ascontiguousarray(
        w6.reshape(128, ITERS * M).astype(cfg.np_data_dt))


def _unscramble(cfg: Cfg, out_core: np.ndarray) -> np.ndarray:
    """(M, NBLK*JH) device output -> (L_K, L) s0 for one batch."""
    o = out_core.reshape(cfg.ql, L_K, NBLK, cfg.jh)  # (j_lo, k, blk, j_hi)
    return np.ascontiguousarray(
        o.transpose(1, 2, 3, 0).reshape(L_K, L)      # j = blk*512+j_hi*QL+j_lo
    )


def _make_in_maps(cfg: Cfg, leafs, q, v):
    dt = cfg.np_data_dt
    return [
        {"leafs": np.ascontiguousarray(leafs[b]).astype(dt),
         "wmat": _build_wmat(cfg, q[b], v[b])}
        for b in range(B)
    ]


def _device_s0(leafs, q, v, cfg: Cfg | None = None) -> np.ndarray:
    """Run the Bass kernel on 8 cores; return s0 (B, L_K, L) float32."""
    global LAST_EXEC_NS, LAST_MEAN_EXEC_NS, LAST_PROFILE
    from concourse.bass_utils import run_bass_kernel_spmd

    cfg = cfg or DEFAULT_CFG
    nc = _get_program(cfg)
    res = run_bass_kernel_spmd(nc, _make_in_maps(cfg, leafs, q, v),
                               list(range(B)), trace=TRACE)
    LAST_EXEC_NS = res.exec_time_ns
    LAST_MEAN_EXEC_NS = res.mean_exec_time_ns
    LAST_PROFILE = res.profile_json
    return np.stack(
        [_unscramble(cfg, res.results[b]["out"]) for b in range(B)])


def _epilogue(s0: np.ndarray, expected: np.ndarray) -> np.float32:
    """Host float64 epilogue: levels, weighted CE, summed — mirrors reference()."""
    s = s0.astype(np.float64)                        # (B, L_K, L) level-0 logits
    labels0 = expected.astype(np.int64)              # (B, L_K)
    n_labels = B * L_K
    depth = int(round(np.log2(L)))
    total = 0.0
    for level in range(depth):
        if level > 0:
            s = 0.5 * (s[..., 0::2] + s[..., 1::2])
        n_cls = L >> level
        labels = labels0 >> level
        counts = np.bincount(labels.reshape(-1), minlength=n_cls).astype(np.float64)
        w = n_labels / (counts + 1e-8)
        w = w / w.sum()
        mx = s.max(axis=-1, keepdims=True)
        logz = np.log(np.exp(s - mx).sum(axis=-1, keepdims=True)) + mx
        logp_y = np.take_along_axis(s - logz, labels[..., None], axis=-1)[..., 0]
        nll = -logp_y                                # (B, L_K)
        wy = w[labels]
        total += ((wy * nll).sum(axis=0) / wy.sum(axis=0)).sum()
    return np.float32(total)


def kernel(q: np.ndarray, v: np.ndarray, expected: np.ndarray,
           leafs: np.ndarray) -> np.ndarray:
    q = np.asarray(q, dtype=np.float32)
    v = np.asarray(v, dtype=np.float32)
    expected = np.asarray(expected)
    leafs = np.asarray(leafs, dtype=np.float32)
    assert q.shape == (B, L_K, D) and leafs.shape == (B, L, D, D)
    s0 = _device_s0(leafs, q, v)
    return np.asarray(_epilogue(s0, expected))


def benchmark(q, v, leafs, iters: int = 20, repeat: int = 1,
              mode: str = "full", cfg: Cfg | None = None):
    """Time the sharded PJRT executable with device-resident inputs.

    Returns (per_call_seconds_list, pipelined_avg_seconds, s0) where s0 is the
    unscrambled result from the last call (for sanity checking).
    """
    import time

    import jax
    import numpy as np_
    from jax.sharding import Mesh, NamedSharding, PartitionSpec
    try:
        from jax.experimental.shard_map import shard_map
    except ImportError:
        from jax.shard_map import shard_map
    from concourse import bass2jax, mybir

    cfg = cfg or DEFAULT_CFG
    bass2jax.install_neuronx_cc_hook()
    nc = (_get_program(cfg) if repeat == 1 and mode == "full"
          else _build_program(cfg, repeat, mode))

    partition_name = (nc.partition_id_tensor.name
                      if nc.partition_id_tensor else None)
    in_names, out_names, out_avals, zero_shapes = [], [], [], []
    for alloc in nc.m.functions[0].allocations:
        if not isinstance(alloc, mybir.MemoryLocationSet):
            continue
        name = alloc.memorylocations[0].name
        if alloc.kind == "ExternalInput":
            if name != partition_name:
                in_names.append(name)
        elif alloc.kind == "ExternalOutput":
            out_names.append(name)
            shape = tuple(alloc.tensor_shape)
            dtype = mybir.dt.np(alloc.dtype)
            out_avals.append(jax.core.ShapedArray(shape, dtype))
            zero_shapes.append((shape, dtype))
    n_params = len(in_names)
    n_outs = len(out_avals)
    all_names = in_names + out_names
    if partition_name is not None:
        all_names = all_names + [partition_name]

    def _body(*args):
        operands = list(args)
        if partition_name is not None:
            operands.append(bass2jax.partition_id_tensor())
        outs = bass2jax._bass_exec_p.bind(
            *operands,
            out_avals=tuple(out_avals),
            in_names=tuple(all_names),
            out_names=tuple(out_names),
            lowering_input_output_aliases=(),
            sim_require_finite=True,
            sim_require_nnan=True,
            nc=nc,
        )
        return tuple(outs)

    devices = jax.devices()[:B]
    mesh = Mesh(np_.asarray(devices), ("core",))
    donate = tuple(range(n_params, n_params + n_outs))
    sharded = jax.jit(
        shard_map(
            _body, mesh=mesh,
            in_specs=(PartitionSpec("core"),) * (n_params + n_outs),
            out_specs=(PartitionSpec("core"),) * n_outs,
            check_rep=False,
        ),
        donate_argnums=donate, keep_unused=True,
    )

    in_maps = _make_in_maps(cfg, leafs, q, v)
    concat_in = [
        np_.concatenate([in_maps[c][nm] for c in range(B)], axis=0)
        for nm in in_names
    ]
    concat_in_dev = [
        jax.device_put(a, NamedSharding(mesh, PartitionSpec("core")))
        for a in concat_in
    ]

    def zeros():
        return [np_.zeros((B * s[0], *s[1:]), d) for s, d in zero_shapes]

    # warmup (includes compile)
    out = sharded(*concat_in_dev, *zeros())
    jax.block_until_ready(out)

    times = []
    last = None
    for _ in range(iters):
        t0 = time.perf_counter()
        out = sharded(*concat_in_dev, *zeros())
        jax.block_until_ready(out)
        times.append(time.perf_counter() - t0)
        last = out

    # pipelined: dispatch all, block once
    t0 = time.perf_counter()
    outs = [sharded(*concat_in_dev, *zeros()) for _ in range(iters)]
    jax.block_until_ready(outs)
    pipelined = (time.perf_counter() - t0) / iters

    oidx = out_names.index("out")
    full = np_.asarray(last[oidx]).reshape(B, cfg.m, NBLK * cfg.jh)
    s0 = np_.stack([_unscramble(cfg, full[b]) for b in range(B)])
    return times, pipelined, s0


def _selftest_numpy():
    """Validate index math (wmat layout + unscramble) in pure numpy."""
    rng = np.random.default_rng(0)
    q = rng.standard_normal((B, L_K, D)).astype(np.float32)
    v = rng.standard_normal((B, L_K, D)).astype(np.float32)
    leafs = rng.standard_normal((1, L, D, D)).astype(np.float32)
    b = 0
    ref = np.einsum('kd,jde,ke->kj', q[b].astype(np.float64),
                    leafs[b].astype(np.float64),
                    v[b].astype(np.float64)) / D
    for cfg in (CFG_A, CFG_B):
        QL, JH, ITERS, M, rp = cfg.ql, cfg.jh, cfg.iters, cfg.m, cfg.rp
        wm = _build_wmat(cfg, q[b], v[b]).astype(np.float64)
        wm = wm.reshape(128, ITERS, M)
        # dtile[p=(jl,r), (jh, it=(d_lo,e))]: leaf j = blk*512 + jh*QL + jl
        lv = leafs[b].reshape(NBLK, JH, QL, 64 // rp, rp, D)
        out = np.zeros((M, NBLK * JH), np.float32)
        for blk in range(NBLK):
            dt_ = lv[blk].transpose(1, 2, 0, 3, 4).reshape(128, JH, ITERS)
            ps = np.einsum('pji,pim->mj', dt_.astype(np.float64), wm)
            out[:, blk * JH:(blk + 1) * JH] = ps.astype(np.float32)
        s0 = _unscramble(cfg, out)
        err = np.abs(s0 - ref).max() / np.abs(ref).max()
        print(f"{cfg.key}: selftest rel err {err:.2e}")
        assert err < 1e-5, (cfg.key, err)
    print("selftest OK")


if __name__ == "__main__":
    _selftest_numpy()



# revision 2
# speedup vs baseline: 3.1373x; 3.1373x over previous
"""MemoryTree oracle loss kernel for 8 Trainium2 NeuronCores.

Strategy
--------
reference() computes, per level l, logits[b,k,n] = q[b,k] @ mem_l[b,n] @ v[b,k] / D
where mem_l is the pairwise-mean tree built from `leafs`. The logit is linear
in the memory matrix and each parent is the *mean* of its children, so level-l
logits are exactly pairwise means of level-0 logits. The only heavy work is the
leaf-level bilinear forms

    s0[b,k,j] = sum_{d,e} leafs[b,j,d,e] * q[b,k,d] * v[b,k,e] / D

one streaming pass over the 512MB `leafs` tensor (memory-bound). The 12-level
log-softmax/NLL/bincount epilogue over 8x4x4096 floats is done on host in
float64 (negligible).

Device mapping (per core = one batch b), v2 contiguous layout:
  - contraction index de = d*64+e in [0,4096): chunk c = de//128, partition
    p = de%128 (CH=32 chunks).
  - leaves j in NBLK=8 blocks of N=512. DRAM data[blk][p][c*N+jj] =
    leafs[b, blk*N+jj].flat[c*128+p]  -> each block's DMA is one dense
    [128, CH*N] transfer (host pre-transposes; fp8 cast on host).
  - per block, PSUM[M, N] accumulates CH chunk-matmuls:
      lhsT = wt[:, c, :] (stationary [128, M]), wt[p,c,m] = q[m,d]*v[m,e]*S/D
      rhs  = data[:, c, :] (moving [128, N])
  - fp8e4m3 + DoubleRow packs 2 chunks per matmul (2x PE rate); 4-way column
    tiling (tile_position) instead runs 4 concurrent matmuls in 32-col strips.
  - weights are scaled by S=32 to clear the fp8 subnormal range; host divides
    the f32 output by S.

Quantizing leafs+weights to fp8e4m3 gives final-loss rel err ~2.5e-4
(gate 2e-2); fp8e3m4 gives ~8e-6; bf16 ~1e-5.

Measured per-pass (slope over For_i hw-loop reps, median of interleaved
rounds; per-core = 16MB fp8 stream at ~460GB/s):
  f8e4_dr_b6 (default) ~34us   (DMA-only floor ~35us, MM-only ~20us)
  f8e3_ct4_b8          ~37us   (4-way col-tiled, 100x better accuracy)
  f8e3                 ~54us   (PE-bound at 1 col/cycle)
  bf16                 ~88us   (2x DMA bytes)
Baseline session: exact-f32 QL-layout kernel measured 327us.
"""

import os
import sys

import numpy as np

for _p in ("/root/.axon_site/_ro/trn_rl_repo", "/opt/trn_rl_repo"):
    if _p not in sys.path and os.path.isdir(_p):
        sys.path.append(_p)

import ml_dtypes

B = 8
L_K = 4
D = 64
L = 4096
DD = D * D
N = 512             # leaves per psum block
NBLK = L // N       # 8
CH = DD // 128      # 32 contraction chunks

_NP_DT = {
    "float8e4": ml_dtypes.float8_e4m3,
    "float8e3": ml_dtypes.float8_e3m4,
    "bfloat16": ml_dtypes.bfloat16,
    "float32r": np.float32,
    "float32": np.float32,
}


class Cfg:
    def __init__(self, key: str, dt_name: str, double_row: bool,
                 scale: float, col_tile: int = 1, bufs: int | None = None):
        self.key = key
        self.dt_name = dt_name
        self.double_row = double_row
        self.col_tile = col_tile
        self.scale = scale
        self.m = 16 if double_row else 4
        self.bufs = bufs or (2 if col_tile == 1 else col_tile + 2)
        assert not (double_row and col_tile > 1)

    @property
    def np_dt(self):
        return _NP_DT[self.dt_name]


CFGS = {
    "f8e4_dr": Cfg("f8e4_dr", "float8e4", True, 32.0),
    "f8e3_ct4": Cfg("f8e3_ct4", "float8e3", False, 32.0, col_tile=4),
    "f8e4_ct4": Cfg("f8e4_ct4", "float8e4", False, 32.0, col_tile=4),
    "f8e3": Cfg("f8e3", "float8e3", False, 32.0),
    "bf16": Cfg("bf16", "bfloat16", False, 1.0),
    "f8e3_ct4_b8": Cfg("f8e3_ct4_b8", "float8e3", False, 32.0, col_tile=4,
                       bufs=8),
    "f8e4_dr_b6": Cfg("f8e4_dr_b6", "float8e4", True, 32.0, bufs=6),
}

DEFAULT_CFG = CFGS[os.environ.get("KERNEL_CFG", "f8e4_dr_b6")]

TRACE = False
LAST_EXEC_NS = None
LAST_MEAN_EXEC_NS = None
LAST_PROFILE = None

_PROGRAMS = {}


def _build_program(cfg: Cfg, repeat: int = 1, mode: str = "full",
                   loop: int = 0):
    """loop>0 wraps the `repeat` unrolled passes in a For_i hardware loop
    executing them `loop` times (for low-noise timing via big in-NEFF
    workloads without instruction blowup)."""
    import contextlib

    import concourse.tile as tile
    from concourse import bacc, mybir

    f32 = mybir.dt.float32
    ddt = getattr(mybir.dt, cfg.dt_name)
    M = cfg.m
    G = cfg.col_tile
    esz = {"float8e4": 1, "float8e3": 1, "bfloat16": 2,
           "float32r": 4, "float32": 4}[cfg.dt_name]

    nc = bacc.Bacc(None, target_bir_lowering=False, debug=False)
    data = nc.declare_dram_parameter("data", [NBLK, 128, CH * N], ddt,
                                     isOutput=False)
    wmat = nc.declare_dram_parameter("wmat", [128, CH * M], ddt,
                                     isOutput=False)
    out = nc.declare_dram_parameter("out", [L_K, L], f32, isOutput=True)

    with tile.TileContext(nc) as tc:
        with (
            tc.tile_pool(name="consts", bufs=1) as consts,
            tc.tile_pool(name="data", bufs=cfg.bufs) as data_pool,
            tc.tile_pool(name="outp", bufs=1) as outp,
            tc.tile_pool(name="psum", bufs=1, space="PSUM") as psum_pool,
        ):
            wt = consts.tile([128, CH * M], ddt)
            nc.sync.dma_start(out=wt[:, :], in_=wmat[:, :])
            wv = wt.rearrange("p (c m) -> p c m", m=M)
            out_sb = outp.tile([L_K, L], f32)

            if G == 1:
                ps_list = [
                    psum_pool.tile([M, N], f32, name=f"ps{i}", tag=f"ps{i}")
                    for i in range(NBLK)
                ]
            else:
                ps_list = [
                    psum_pool.tile([128, N], f32, name=f"ps{i}", tag=f"ps{i}")
                    for i in range(NBLK // G)
                ]

            fixed_dtile = None
            if mode == "mm":
                fixed_dtile = consts.tile([128, CH * N], ddt)
                nc.sync.dma_start(out=fixed_dtile[:, :], in_=data[0, :, :])

            def load(blk):
                if mode == "mm":
                    return fixed_dtile
                dtile = data_pool.tile([128, CH * N], ddt)
                nc.sync.dma_start(out=dtile[:, :], in_=data[blk, :, :])
                return dtile

            def dep_copy(blk, dtile):
                nc.vector.tensor_copy(
                    out=out_sb[0:1, blk * N:blk * N + 1],
                    in_=dtile[0:1, 0:4 // esz].bitcast(f32)
                    if ddt != f32 else dtile[0:1, 0:1],
                )

            loop_ctx = (tc.For_i(0, loop, 1)
                        if loop else contextlib.nullcontext())
            with loop_ctx:
                self_repeat_body(cfg, repeat, mode, nc, mybir, G, M,
                                 load, dep_copy, wv, ps_list, out_sb)

            nc.sync.dma_start(out=out[:, :], in_=out_sb[:, :])

    nc.compile()
    return nc


def self_repeat_body(cfg, repeat, mode, nc, mybir, G, M,
                     load, dep_copy, wv, ps_list, out_sb):
    N_ = N
    for rep in range(repeat):
                if G == 1:
                    for blk in range(NBLK):
                        dtile = load(blk)
                        if mode == "dma":
                            dep_copy(blk, dtile)
                            continue
                        dv = dtile.rearrange("p (c j) -> p c j", j=N)
                        ps = ps_list[blk]
                        if cfg.double_row:
                            nmm = CH // 2
                            for cc in range(nmm):
                                nc.tensor.matmul(
                                    out=ps[:, :],
                                    lhsT=wv[:, 2 * cc:2 * cc + 2, :],
                                    rhs=dv[:, 2 * cc:2 * cc + 2, :],
                                    start=(cc == 0),
                                    stop=(cc == nmm - 1),
                                    perf_mode=mybir.MatmulPerfMode.DoubleRow,
                                )
                        else:
                            for c in range(CH):
                                nc.tensor.matmul(
                                    out=ps[:, :],
                                    lhsT=wv[:, c, :],
                                    rhs=dv[:, c, :],
                                    start=(c == 0),
                                    stop=(c == CH - 1),
                                )
                        nc.vector.tensor_copy(
                            out=out_sb[:, blk * N:(blk + 1) * N],
                            in_=ps[0:L_K, :],
                        )
                else:
                    for sb in range(NBLK // G):
                        dtiles = [load(sb * G + g) for g in range(G)]
                        if mode == "dma":
                            for g in range(G):
                                dep_copy(sb * G + g, dtiles[g])
                            continue
                        dvs = [t.rearrange("p (c j) -> p c j", j=N)
                               for t in dtiles]
                        ps = ps_list[sb]
                        for c in range(CH):
                            for g in range(G):
                                nc.tensor.matmul(
                                    out=ps[32 * g:32 * g + M, :],
                                    lhsT=wv[:, c, :],
                                    rhs=dvs[g][:, c, :],
                                    start=(c == 0),
                                    stop=(c == CH - 1),
                                    tile_position=(0, 32 * g),
                                )
                        for g in range(G):
                            nc.vector.tensor_copy(
                                out=out_sb[:, (sb * G + g) * N:
                                           (sb * G + g + 1) * N],
                                in_=ps[32 * g:32 * g + L_K, :],
                            )

            nc.sync.dma_start(out=out[:, :], in_=out_sb[:, :])

    nc.compile()
    return nc


def _get_program(cfg: Cfg):
    if cfg.key not in _PROGRAMS:
        _PROGRAMS[cfg.key] = _build_program(cfg)
    return _PROGRAMS[cfg.key]


def _prep_data(cfg: Cfg, leafs_b: np.ndarray) -> np.ndarray:
    """leafs_b (L, D, D) f32 -> (NBLK, 128, CH*N) in cfg dtype."""
    a = leafs_b.reshape(NBLK, N, CH, 128)
    a = np.ascontiguousarray(a.transpose(0, 3, 2, 1))  # blk, p, c, jj
    return a.reshape(NBLK, 128, CH * N).astype(cfg.np_dt)


def _prep_wmat(cfg: Cfg, q_b: np.ndarray, v_b: np.ndarray) -> np.ndarray:
    """(L_K, D) x2 -> (128, CH*M): wt[p, c*M+m] = qv[m, c*128+p]*S/D."""
    qv = (q_b[:, :, None].astype(np.float64) * v_b[:, None, :].astype(np.float64)
          * (cfg.scale / D))
    w = np.zeros((cfg.m, DD), np.float32)
    w[:L_K] = qv.reshape(L_K, DD).astype(np.float32)
    w = w.reshape(cfg.m, CH, 128)
    w = np.ascontiguousarray(w.transpose(2, 1, 0))  # p, c, m
    return w.reshape(128, CH * cfg.m).astype(cfg.np_dt)


def _make_in_maps(cfg: Cfg, leafs, q, v):
    return [
        {"data": _prep_data(cfg, leafs[b]),
         "wmat": _prep_wmat(cfg, q[b], v[b])}
        for b in range(B)
    ]


def _device_s0(leafs, q, v, cfg: Cfg | None = None) -> np.ndarray:
    """Run the Bass kernel on 8 cores; return s0 (B, L_K, L) float32."""
    global LAST_EXEC_NS, LAST_MEAN_EXEC_NS, LAST_PROFILE
    from concourse.bass_utils import run_bass_kernel_spmd

    cfg = cfg or DEFAULT_CFG
    nc = _get_program(cfg)
    res = run_bass_kernel_spmd(nc, _make_in_maps(cfg, leafs, q, v),
                               list(range(B)), trace=TRACE)
    LAST_EXEC_NS = res.exec_time_ns
    LAST_MEAN_EXEC_NS = res.mean_exec_time_ns
    LAST_PROFILE = res.profile_json
    return np.stack([res.results[b]["out"] for b in range(B)]) / cfg.scale


def _epilogue(s0: np.ndarray, expected: np.ndarray) -> np.float32:
    """Host float64 epilogue: levels, weighted CE, summed — mirrors reference()."""
    s = s0.astype(np.float64)
    labels0 = expected.astype(np.int64)
    n_labels = B * L_K
    depth = int(round(np.log2(L)))
    total = 0.0
    for level in range(depth):
        if level > 0:
            s = 0.5 * (s[..., 0::2] + s[..., 1::2])
        n_cls = L >> level
        labels = labels0 >> level
        counts = np.bincount(labels.reshape(-1), minlength=n_cls).astype(np.float64)
        w = n_labels / (counts + 1e-8)
        w = w / w.sum()
        mx = s.max(axis=-1, keepdims=True)
        logz = np.log(np.exp(s - mx).sum(axis=-1, keepdims=True)) + mx
        logp_y = np.take_along_axis(s - logz, labels[..., None], axis=-1)[..., 0]
        nll = -logp_y
        wy = w[labels]
        total += ((wy * nll).sum(axis=0) / wy.sum(axis=0)).sum()
    return np.float32(total)


def kernel(q: np.ndarray, v: np.ndarray, expected: np.ndarray,
           leafs: np.ndarray) -> np.ndarray:
    q = np.asarray(q, dtype=np.float32)
    v = np.asarray(v, dtype=np.float32)
    expected = np.asarray(expected)
    leafs = np.asarray(leafs, dtype=np.float32)
    assert q.shape == (B, L_K, D) and leafs.shape == (B, L, D, D)
    s0 = _device_s0(leafs, q, v)
    return np.asarray(_epilogue(s0, expected))


def make_runner(q, v, leafs, cfg: Cfg | None = None, repeat: int = 1,
                mode: str = "full", loop: int = 0):
    cfg = cfg or DEFAULT_CFG
    nc = (_get_program(cfg) if repeat == 1 and mode == "full" and not loop
          else _build_program(cfg, repeat, mode, loop))
    return _runner_from_nc(nc, cfg, q, v, leafs)


def _runner_from_nc(nc, cfg: Cfg, q, v, leafs):
    """Warm a sharded 8-core executable with device-resident inputs;
    return fn(iters) -> pipelined average seconds per call."""
    import time

    import jax
    import numpy as np_
    from jax.sharding import Mesh, NamedSharding, PartitionSpec
    try:
        from jax.experimental.shard_map import shard_map
    except ImportError:
        from jax.shard_map import shard_map
    from concourse import bass2jax, mybir

    bass2jax.install_neuronx_cc_hook()

    partition_name = (nc.partition_id_tensor.name
                      if nc.partition_id_tensor else None)
    in_names, out_names, out_avals, zero_shapes = [], [], [], []
    for alloc in nc.m.functions[0].allocations:
        if not isinstance(alloc, mybir.MemoryLocationSet):
            continue
        name = alloc.memorylocations[0].name
        if alloc.kind == "ExternalInput":
            if name != partition_name:
                in_names.append(name)
        elif alloc.kind == "ExternalOutput":
            out_names.append(name)
            shape = tuple(alloc.tensor_shape)
            dtype = mybir.dt.np(alloc.dtype)
            out_avals.append(jax.core.ShapedArray(shape, dtype))
            zero_shapes.append((shape, dtype))
    n_params = len(in_names)
    n_outs = len(out_avals)
    all_names = in_names + out_names
    if partition_name is not None:
        all_names = all_names + [partition_name]

    def _body(*args):
        operands = list(args)
        if partition_name is not None:
            operands.append(bass2jax.partition_id_tensor())
        outs = bass2jax._bass_exec_p.bind(
            *operands,
            out_avals=tuple(out_avals),
            in_names=tuple(all_names),
            out_names=tuple(out_names),
            lowering_input_output_aliases=(),
            sim_require_finite=True,
            sim_require_nnan=True,
            nc=nc,
        )
        return tuple(outs)

    devices = jax.devices()[:B]
    mesh = Mesh(np_.asarray(devices), ("core",))
    donate = tuple(range(n_params, n_params + n_outs))
    sharded = jax.jit(
        shard_map(
            _body, mesh=mesh,
            in_specs=(PartitionSpec("core"),) * (n_params + n_outs),
            out_specs=(PartitionSpec("core"),) * n_outs,
            check_rep=False,
        ),
        donate_argnums=donate, keep_unused=True,
    )

    in_maps = _make_in_maps(cfg, leafs, q, v)
    concat_in = [
        np_.concatenate([in_maps[c][nm] for c in range(B)], axis=0)
        for nm in in_names
    ]
    concat_in_dev = [
        jax.device_put(a, NamedSharding(mesh, PartitionSpec("core")))
        for a in concat_in
    ]

    def zeros():
        return [np_.zeros((B * s[0], *s[1:]), d) for s, d in zero_shapes]

    out = sharded(*concat_in_dev, *zeros())
    jax.block_until_ready(out)

    def run(iters: int = 20) -> float:
        t0 = time.perf_counter()
        outs = [sharded(*concat_in_dev, *zeros()) for _ in range(iters)]
        jax.block_until_ready(outs)
        return (time.perf_counter() - t0) / iters

    return run


def _selftest_numpy():
    """Validate index math (layout + weights) in pure numpy."""
    rng = np.random.default_rng(0)
    q = rng.standard_normal((L_K, D)).astype(np.float32)
    v = rng.standard_normal((L_K, D)).astype(np.float32)
    leafs = rng.standard_normal((L, D, D)).astype(np.float32)
    ref = np.einsum('kd,jde,ke->kj', q.astype(np.float64),
                    leafs.astype(np.float64), v.astype(np.float64)) / D
    for key, cfg in CFGS.items():
        data_b = _prep_data(cfg, leafs).astype(np.float64)
        w = _prep_wmat(cfg, q, v).astype(np.float64).reshape(128, CH, cfg.m)
        out = np.zeros((L_K, L), np.float32)
        for blk in range(NBLK):
            dv = data_b[blk].reshape(128, CH, N)
            ps = np.einsum('pcm,pcj->mj', w, dv)
            out[:, blk * N:(blk + 1) * N] = ps[:L_K].astype(np.float32)
        s0 = out / cfg.scale
        err = np.abs(s0 - ref).max() / np.abs(ref).max()
        print(f"{key}: selftest rel err {err:.2e}")
    print("selftest OK")


if __name__ == "__main__":
    _selftest_numpy()


# revision 3
# speedup vs baseline: 3.1440x; 1.0021x over previous
"""MemoryTree oracle loss kernel for 8 Trainium2 NeuronCores.

Strategy
--------
reference() computes, per level l, logits[b,k,n] = q[b,k] @ mem_l[b,n] @ v[b,k] / D
where mem_l is the pairwise-mean tree built from `leafs`. The logit is linear
in the memory matrix and each parent is the *mean* of its children, so level-l
logits are exactly pairwise means of level-0 logits. The only heavy work is the
leaf-level bilinear forms

    s0[b,k,j] = sum_{d,e} leafs[b,j,d,e] * q[b,k,d] * v[b,k,e] / D

one streaming pass over the 512MB `leafs` tensor (memory-bound). The 12-level
log-softmax/NLL/bincount epilogue over 8x4x4096 floats is done on host in
float64 (negligible).

Device mapping (per core = one batch b), v2 contiguous layout:
  - contraction index de = d*64+e in [0,4096): chunk c = de//128, partition
    p = de%128 (CH=32 chunks).
  - leaves j in NBLK=8 blocks of N=512. DRAM data[blk][p][c*N+jj] =
    leafs[b, blk*N+jj].flat[c*128+p]  -> each block's DMA is one dense
    [128, CH*N] transfer (host pre-transposes; fp8 cast on host).
  - per block, PSUM[M, N] accumulates CH chunk-matmuls:
      lhsT = wt[:, c, :] (stationary [128, M]), wt[p,c,m] = q[m,d]*v[m,e]*S/D
      rhs  = data[:, c, :] (moving [128, N])
  - fp8e4m3 + DoubleRow packs 2 chunks per matmul (2x PE rate); 4-way column
    tiling (tile_position) instead runs 4 concurrent matmuls in 32-col strips.
  - weights are scaled by S=32 to clear the fp8 subnormal range; host divides
    the f32 output by S.

Quantizing leafs+weights to fp8e4m3 gives final-loss rel err ~2.5e-4
(gate 2e-2); fp8e3m4 gives ~8e-6; bf16 ~1e-5.

Measured per-pass (slope over For_i hw-loop reps, median of interleaved
rounds; per-core = 16MB fp8 stream at ~460GB/s):
  f8e4_dr_b6 (default) ~34us   (DMA-only floor ~35us, MM-only ~20us)
  f8e3_ct4_b8          ~37us   (4-way col-tiled, 100x better accuracy)
  f8e3                 ~54us   (PE-bound at 1 col/cycle)
  bf16                 ~88us   (2x DMA bytes)
Baseline session: exact-f32 QL-layout kernel measured 327us.

Roofline notes: the pure-DMA floor (~34.6us for 16MB) = 463GB/s = the
16-SDMA-engine aggregate (16 x 27GiB/s); a second HWDGE ring (nc.scalar)
or SWDGE (nc.gpsimd) adds nothing (engines are shared), so the kernel is
DMA-bound at the fabric rate with full PE/DMA overlap. Sustained >40ms
bursts throttle to ~370GB/s (HBM-side limit) - per-pass measures ~42us
with loop counts >300; test.py uses ~25ms bursts (loop=170, rep=4).
"""

import os
import sys

import numpy as np

for _p in ("/root/.axon_site/_ro/trn_rl_repo", "/opt/trn_rl_repo"):
    if _p not in sys.path and os.path.isdir(_p):
        sys.path.append(_p)

import ml_dtypes

B = 8
L_K = 4
D = 64
L = 4096
DD = D * D
N = 512             # leaves per psum block
NBLK = L // N       # 8
CH = DD // 128      # 32 contraction chunks

_NP_DT = {
    "float8e4": ml_dtypes.float8_e4m3,
    "float8e3": ml_dtypes.float8_e3m4,
    "bfloat16": ml_dtypes.bfloat16,
    "float32r": np.float32,
    "float32": np.float32,
}


class Cfg:
    def __init__(self, key: str, dt_name: str, double_row: bool,
                 scale: float, col_tile: int = 1, bufs: int | None = None,
                 rings: int = 1):
        self.key = key
        self.dt_name = dt_name
        self.double_row = double_row
        self.col_tile = col_tile
        self.scale = scale
        self.m = 16 if double_row else 4
        self.bufs = bufs or (2 if col_tile == 1 else col_tile + 2)
        self.rings = rings
        assert not (double_row and col_tile > 1)

    @property
    def np_dt(self):
        return _NP_DT[self.dt_name]


CFGS = {
    "f8e4_dr": Cfg("f8e4_dr", "float8e4", True, 32.0),
    "f8e3_ct4": Cfg("f8e3_ct4", "float8e3", False, 32.0, col_tile=4),
    "f8e4_ct4": Cfg("f8e4_ct4", "float8e4", False, 32.0, col_tile=4),
    "f8e3": Cfg("f8e3", "float8e3", False, 32.0),
    "bf16": Cfg("bf16", "bfloat16", False, 1.0),
    "f8e3_ct4_b8": Cfg("f8e3_ct4_b8", "float8e3", False, 32.0, col_tile=4,
                       bufs=8),
    "f8e4_dr_b6": Cfg("f8e4_dr_b6", "float8e4", True, 32.0, bufs=6),
    "f8e4_dr_r2": Cfg("f8e4_dr_r2", "float8e4", True, 32.0, bufs=6, rings=2),
    "f8e4_dr_r3": Cfg("f8e4_dr_r3", "float8e4", True, 32.0, bufs=6, rings=3),
    "f8e3_ct4_r2": Cfg("f8e3_ct4_r2", "float8e3", False, 32.0, col_tile=4,
                       bufs=8, rings=2),
}

DEFAULT_CFG = CFGS[os.environ.get("KERNEL_CFG", "f8e4_dr_b6")]

TRACE = False
LAST_EXEC_NS = None
LAST_MEAN_EXEC_NS = None
LAST_PROFILE = None

_PROGRAMS = {}


def _build_program(cfg: Cfg, repeat: int = 1, mode: str = "full",
                   loop: int = 0, staggered: bool = False):
    """loop>0 wraps the `repeat` unrolled passes in a For_i hardware loop
    executing them `loop` times (for low-noise timing via big in-NEFF
    workloads without instruction blowup)."""
    import contextlib

    import concourse.tile as tile
    from concourse import bacc, mybir

    f32 = mybir.dt.float32
    ddt = getattr(mybir.dt, cfg.dt_name)
    M = cfg.m
    G = cfg.col_tile
    esz = {"float8e4": 1, "float8e3": 1, "bfloat16": 2,
           "float32r": 4, "float32": 4}[cfg.dt_name]

    nc = bacc.Bacc(None, target_bir_lowering=False, debug=False)
    data = nc.declare_dram_parameter("data", [NBLK, 128, CH * N], ddt,
                                     isOutput=False)
    wmat = nc.declare_dram_parameter("wmat", [128, CH * M], ddt,
                                     isOutput=False)
    out = nc.declare_dram_parameter("out", [L_K, L], f32, isOutput=True)

    with tile.TileContext(nc) as tc:
        with (
            tc.tile_pool(name="consts", bufs=1) as consts,
            tc.tile_pool(name="data", bufs=cfg.bufs) as data_pool,
            tc.tile_pool(name="outp", bufs=1) as outp,
            tc.tile_pool(name="psum", bufs=1, space="PSUM") as psum_pool,
        ):
            wt = consts.tile([128, CH * M], ddt)
            nc.sync.dma_start(out=wt[:, :], in_=wmat[:, :])
            wv = wt.rearrange("p (c m) -> p c m", m=M)
            out_sb = outp.tile([L_K, L], f32)

            if G == 1:
                ps_list = [
                    psum_pool.tile([M, N], f32, name=f"ps{i}", tag=f"ps{i}")
                    for i in range(NBLK)
                ]
            else:
                ps_list = [
                    psum_pool.tile([128, N], f32, name=f"ps{i}", tag=f"ps{i}")
                    for i in range(NBLK // G)
                ]

            fixed_dtile = None
            if mode == "mm":
                fixed_dtile = consts.tile([128, CH * N], ddt)
                nc.sync.dma_start(out=fixed_dtile[:, :], in_=data[0, :, :])

            dma_engs = [nc.sync, nc.scalar, nc.gpsimd][:cfg.rings]

            def load(blk):
                if mode == "mm":
                    return fixed_dtile
                dtile = data_pool.tile([128, CH * N], ddt)
                eng = dma_engs[blk % len(dma_engs)]
                eng.dma_start(out=dtile[:, :], in_=data[blk, :, :])
                return dtile

            def dep_copy(blk, dtile):
                nc.vector.tensor_copy(
                    out=out_sb[0:1, blk * N:blk * N + 1],
                    in_=dtile[0:1, 0:4 // esz].bitcast(f32)
                    if ddt != f32 else dtile[0:1, 0:1],
                )

            loop_ctx = (tc.For_i(0, loop, 1)
                        if loop else contextlib.nullcontext())
            with loop_ctx:
                self_repeat_body(cfg, repeat, mode, nc, mybir, G, M,
                                 load, dep_copy, wv, ps_list, out_sb)

            nc.sync.dma_start(out=out[:, :], in_=out_sb[:, :])

    nc.compile()
    return nc


def self_repeat_body(cfg, repeat, mode, nc, mybir, G, M,
                     load, dep_copy, wv, ps_list, out_sb):
    N_ = N
    for rep in range(repeat):
                if G == 1:
                    for blk in range(NBLK):
                        dtile = load(blk)
                        if mode == "dma":
                            dep_copy(blk, dtile)
                            continue
                        dv = dtile.rearrange("p (c j) -> p c j", j=N)
                        ps = ps_list[blk]
                        if cfg.double_row:
                            nmm = CH // 2
                            for cc in range(nmm):
                                nc.tensor.matmul(
                                    out=ps[:, :],
                                    lhsT=wv[:, 2 * cc:2 * cc + 2, :],
                                    rhs=dv[:, 2 * cc:2 * cc + 2, :],
                                    start=(cc == 0),
                                    stop=(cc == nmm - 1),
                                    perf_mode=mybir.MatmulPerfMode.DoubleRow,
                                )
                        else:
                            for c in range(CH):
                                nc.tensor.matmul(
                                    out=ps[:, :],
                                    lhsT=wv[:, c, :],
                                    rhs=dv[:, c, :],
                                    start=(c == 0),
                                    stop=(c == CH - 1),
                                )
                        nc.vector.tensor_copy(
                            out=out_sb[:, blk * N:(blk + 1) * N],
                            in_=ps[0:L_K, :],
                        )
                else:
                    for sb in range(NBLK // G):
                        dtiles = [load(sb * G + g) for g in range(G)]
                        if mode == "dma":
                            for g in range(G):
                                dep_copy(sb * G + g, dtiles[g])
                            continue
                        dvs = [t.rearrange("p (c j) -> p c j", j=N)
                               for t in dtiles]
                        ps = ps_list[sb]
                        for c in range(CH):
                            for g in range(G):
                                nc.tensor.matmul(
                                    out=ps[32 * g:32 * g + M, :],
                                    lhsT=wv[:, c, :],
                                    rhs=dvs[g][:, c, :],
                                    start=(c == 0),
                                    stop=(c == CH - 1),
                                    tile_position=(0, 32 * g),
                                )
                        for g in range(G):
                            nc.vector.tensor_copy(
                                out=out_sb[:, (sb * G + g) * N:
                                           (sb * G + g + 1) * N],
                                in_=ps[32 * g:32 * g + L_K, :],
                            )

            nc.sync.dma_start(out=out[:, :], in_=out_sb[:, :])

    nc.compile()
    return nc


def _get_program(cfg: Cfg):
    if cfg.key not in _PROGRAMS:
        _PROGRAMS[cfg.key] = _build_program(cfg)
    return _PROGRAMS[cfg.key]


def _prep_data(cfg: Cfg, leafs_b: np.ndarray) -> np.ndarray:
    """leafs_b (L, D, D) f32 -> (NBLK, 128, CH*N) in cfg dtype."""
    a = leafs_b.reshape(NBLK, N, CH, 128)
    a = np.ascontiguousarray(a.transpose(0, 3, 2, 1))  # blk, p, c, jj
    return a.reshape(NBLK, 128, CH * N).astype(cfg.np_dt)


def _prep_wmat(cfg: Cfg, q_b: np.ndarray, v_b: np.ndarray) -> np.ndarray:
    """(L_K, D) x2 -> (128, CH*M): wt[p, c*M+m] = qv[m, c*128+p]*S/D."""
    qv = (q_b[:, :, None].astype(np.float64) * v_b[:, None, :].astype(np.float64)
          * (cfg.scale / D))
    w = np.zeros((cfg.m, DD), np.float32)
    w[:L_K] = qv.reshape(L_K, DD).astype(np.float32)
    w = w.reshape(cfg.m, CH, 128)
    w = np.ascontiguousarray(w.transpose(2, 1, 0))  # p, c, m
    return w.reshape(128, CH * cfg.m).astype(cfg.np_dt)


def _make_in_maps(cfg: Cfg, leafs, q, v):
    return [
        {"data": _prep_data(cfg, leafs[b]),
         "wmat": _prep_wmat(cfg, q[b], v[b])}
        for b in range(B)
    ]


def _device_s0(leafs, q, v, cfg: Cfg | None = None) -> np.ndarray:
    """Run the Bass kernel on 8 cores; return s0 (B, L_K, L) float32."""
    global LAST_EXEC_NS, LAST_MEAN_EXEC_NS, LAST_PROFILE
    from concourse.bass_utils import run_bass_kernel_spmd

    cfg = cfg or DEFAULT_CFG
    nc = _get_program(cfg)
    res = run_bass_kernel_spmd(nc, _make_in_maps(cfg, leafs, q, v),
                               list(range(B)), trace=TRACE)
    LAST_EXEC_NS = res.exec_time_ns
    LAST_MEAN_EXEC_NS = res.mean_exec_time_ns
    LAST_PROFILE = res.profile_json
    return np.stack([res.results[b]["out"] for b in range(B)]) / cfg.scale


def _epilogue(s0: np.ndarray, expected: np.ndarray) -> np.float32:
    """Host float64 epilogue: levels, weighted CE, summed — mirrors reference()."""
    s = s0.astype(np.float64)
    labels0 = expected.astype(np.int64)
    n_labels = B * L_K
    depth = int(round(np.log2(L)))
    total = 0.0
    for level in range(depth):
        if level > 0:
            s = 0.5 * (s[..., 0::2] + s[..., 1::2])
        n_cls = L >> level
        labels = labels0 >> level
        counts = np.bincount(labels.reshape(-1), minlength=n_cls).astype(np.float64)
        w = n_labels / (counts + 1e-8)
        w = w / w.sum()
        mx = s.max(axis=-1, keepdims=True)
        logz = np.log(np.exp(s - mx).sum(axis=-1, keepdims=True)) + mx
        logp_y = np.take_along_axis(s - logz, labels[..., None], axis=-1)[..., 0]
        nll = -logp_y
        wy = w[labels]
        total += ((wy * nll).sum(axis=0) / wy.sum(axis=0)).sum()
    return np.float32(total)


def kernel(q: np.ndarray, v: np.ndarray, expected: np.ndarray,
           leafs: np.ndarray) -> np.ndarray:
    q = np.asarray(q, dtype=np.float32)
    v = np.asarray(v, dtype=np.float32)
    expected = np.asarray(expected)
    leafs = np.asarray(leafs, dtype=np.float32)
    assert q.shape == (B, L_K, D) and leafs.shape == (B, L, D, D)
    s0 = _device_s0(leafs, q, v)
    return np.asarray(_epilogue(s0, expected))


def make_runner(q, v, leafs, cfg: Cfg | None = None, repeat: int = 1,
                mode: str = "full", loop: int = 0):
    cfg = cfg or DEFAULT_CFG
    nc = (_get_program(cfg) if repeat == 1 and mode == "full" and not loop
          else _build_program(cfg, repeat, mode, loop))
    return _runner_from_nc(nc, cfg, q, v, leafs)


def _runner_from_nc(nc, cfg: Cfg, q, v, leafs):
    """Warm a sharded 8-core executable with device-resident inputs;
    return fn(iters) -> pipelined average seconds per call."""
    import time

    import jax
    import numpy as np_
    from jax.sharding import Mesh, NamedSharding, PartitionSpec
    try:
        from jax.experimental.shard_map import shard_map
    except ImportError:
        from jax.shard_map import shard_map
    from concourse import bass2jax, mybir

    bass2jax.install_neuronx_cc_hook()

    partition_name = (nc.partition_id_tensor.name
                      if nc.partition_id_tensor else None)
    in_names, out_names, out_avals, zero_shapes = [], [], [], []
    for alloc in nc.m.functions[0].allocations:
        if not isinstance(alloc, mybir.MemoryLocationSet):
            continue
        name = alloc.memorylocations[0].name
        if alloc.kind == "ExternalInput":
            if name != partition_name:
                in_names.append(name)
        elif alloc.kind == "ExternalOutput":
            out_names.append(name)
            shape = tuple(alloc.tensor_shape)
            dtype = mybir.dt.np(alloc.dtype)
            out_avals.append(jax.core.ShapedArray(shape, dtype))
            zero_shapes.append((shape, dtype))
    n_params = len(in_names)
    n_outs = len(out_avals)
    all_names = in_names + out_names
    if partition_name is not None:
        all_names = all_names + [partition_name]

    def _body(*args):
        operands = list(args)
        if partition_name is not None:
            operands.append(bass2jax.partition_id_tensor())
        outs = bass2jax._bass_exec_p.bind(
            *operands,
            out_avals=tuple(out_avals),
            in_names=tuple(all_names),
            out_names=tuple(out_names),
            lowering_input_output_aliases=(),
            sim_require_finite=True,
            sim_require_nnan=True,
            nc=nc,
        )
        return tuple(outs)

    devices = jax.devices()[:B]
    mesh = Mesh(np_.asarray(devices), ("core",))
    donate = tuple(range(n_params, n_params + n_outs))
    sharded = jax.jit(
        shard_map(
            _body, mesh=mesh,
            in_specs=(PartitionSpec("core"),) * (n_params + n_outs),
            out_specs=(PartitionSpec("core"),) * n_outs,
            check_rep=False,
        ),
        donate_argnums=donate, keep_unused=True,
    )

    in_maps = _make_in_maps(cfg, leafs, q, v)
    concat_in = [
        np_.concatenate([in_maps[c][nm] for c in range(B)], axis=0)
        for nm in in_names
    ]
    concat_in_dev = [
        jax.device_put(a, NamedSharding(mesh, PartitionSpec("core")))
        for a in concat_in
    ]

    def zeros():
        return [np_.zeros((B * s[0], *s[1:]), d) for s, d in zero_shapes]

    out = sharded(*concat_in_dev, *zeros())
    jax.block_until_ready(out)

    def run(iters: int = 20) -> float:
        t0 = time.perf_counter()
        outs = [sharded(*concat_in_dev, *zeros()) for _ in range(iters)]
        jax.block_until_ready(outs)
        return (time.perf_counter() - t0) / iters

    return run


def _selftest_numpy():
    """Validate index math (layout + weights) in pure numpy."""
    rng = np.random.default_rng(0)
    q = rng.standard_normal((L_K, D)).astype(np.float32)
    v = rng.standard_normal((L_K, D)).astype(np.float32)
    leafs = rng.standard_normal((L, D, D)).astype(np.float32)
    ref = np.einsum('kd,jde,ke->kj', q.astype(np.float64),
                    leafs.astype(np.float64), v.astype(np.float64)) / D
    for key, cfg in CFGS.items():
        data_b = _prep_data(cfg, leafs).astype(np.float64)
        w = _prep_wmat(cfg, q, v).astype(np.float64).reshape(128, CH, cfg.m)
        out = np.zeros((L_K, L), np.float32)
        for blk in range(NBLK):
            dv = data_b[blk].reshape(128, CH, N)
            ps = np.einsum('pcm,pcj->mj', w, dv)
            out[:, blk * N:(blk + 1) * N] = ps[:L_K].astype(np.float32)
        s0 = out / cfg.scale
        err = np.abs(s0 - ref).max() / np.abs(ref).max()
        print(f"{key}: selftest rel err {err:.2e}")
    print("selftest OK")


if __name__ == "__main__":
    _selftest_numpy()
